# revision 8
# baseline (speedup 1.0000x reference)
"""DetectionLayer (YOLOv2-style head) Trainium2 Bass kernel.

Input:  b_x [512, 82320] f32  (= [512, 28, 28, 105]; 105 = 5*(4+1) + 80)
Output: (b_pred_loc [512, 3920, 4], b_cls [512, 3920, 80], b_conf [512, 3920])

Sharding: pure data parallel over batch; 64 examples per NeuronCore x 8 cores.
Per-core layout: 64*784 = 50176 cells viewed as 28 tiles of [128 partitions,
14 cells, 105 ch]; every DMA row is contiguous in DRAM (>= 1120 B).
"""

import numpy as np

import concourse.bass as bass
import concourse.bacc as bacc
import concourse.tile as tile
from concourse import mybir
from concourse.bass_utils import run_bass_kernel_spmd

SIDE = 28
NUM = 5
CLASSES = 80
COORDS = 4
BATCH = 512
NCORES = 8
CH = NUM * (COORDS + 1)            # 25
FEAT = SIDE * SIDE * (CH + CLASSES)  # 82320
BPC = BATCH // NCORES              # 64 examples per core
CELLS = BPC * SIDE * SIDE          # 50176 cells per core
P = 128                            # SBUF partitions
K = 14                             # cells per partition per tile
NT = CELLS // (P * K)              # 28 tiles per core
BOXES = SIDE * SIDE * NUM          # 3920

_F32 = mybir.dt.float32
_AF = mybir.ActivationFunctionType
_OP = mybir.AluOpType

_CACHE = {}


def _grid_const() -> np.ndarray:
    # gxy[p, t, j, a, 0] = (cell col)/SIDE, [..., 1] = (cell row)/SIDE for the
    # cell at flat index t*(P*K) + p*K + j (cell index within an example
    # repeats every SIDE*SIDE). Replicated over the NUM anchors so on-device
    # access patterns stay contiguous (the BIR verifier caps compute-op APs
    # at 3 dims, which rules out stride-0 anchor broadcast here).
    n = np.arange(CELLS, dtype=np.int64)
    s = n % (SIDE * SIDE)
    gx = (s % SIDE).astype(np.float32) / SIDE
    gy = (s // SIDE).astype(np.float32) / SIDE
    gxy = np.stack([gx, gy], axis=-1)          # [CELLS, 2]
    gxy = np.repeat(gxy[:, None, :], NUM, axis=1)  # [CELLS, NUM, 2]
    gxy = gxy.reshape(NT, P, K, NUM, 2).transpose(1, 0, 2, 3, 4)
    return np.ascontiguousarray(gxy.reshape(P, NT * K * NUM * 2))


def _build(repeat: int = 1) -> bass.Bass:
    nc = bacc.Bacc("TRN2", target_bir_lowering=False)

    x = nc.dram_tensor("x", [BPC, FEAT], _F32, kind="ExternalInput")
    g = nc.dram_tensor("g", [P, NT * K * NUM * 2], _F32, kind="ExternalInput")
    loc = nc.dram_tensor("loc", [BPC, BOXES, COORDS], _F32, kind="ExternalOutput")
    cls = nc.dram_tensor("cls", [BPC, BOXES, CLASSES], _F32, kind="ExternalOutput")
    conf = nc.dram_tensor("conf", [BPC, BOXES], _F32, kind="ExternalOutput")

    xv = x[:, :].flatten().rearrange("(t p k c) -> t p k c", t=NT, p=P, k=K)
    locv = loc[:, :, :].flatten().rearrange(
        "(t p k a c) -> t p k a c", t=NT, p=P, k=K, a=NUM
    )
    clsv = cls[:, :, :].flatten().rearrange(
        "(t p k a c) -> t p k a c", t=NT, p=P, k=K, a=NUM
    )
    confv = conf[:, :].flatten().rearrange("(t p k a) -> t p k a", t=NT, p=P, k=K)
    gv = g[:, :].rearrange("p (t k a c) -> p t k a c", t=NT, k=K, a=NUM)

    with tile.TileContext(nc) as tc:
        with (
            tc.tile_pool(name="const", bufs=1) as constp,
            tc.tile_pool(name="io", bufs=3) as iop,
            tc.tile_pool(name="work", bufs=3) as workp,
            tc.tile_pool(name="clsout", bufs=3) as clsp,
            tc.tile_pool(name="locout", bufs=3) as locp,
        ):
            gsb = constp.tile([P, NT, K, NUM, 2], _F32)
            nc.sync.dma_start(out=gsb, in_=gv)

            for t in [t for _ in range(repeat) for t in range(NT)]:
                xt = iop.tile([P, K, CH + CLASSES], _F32)
                nc.sync.dma_start(out=xt, in_=xv[t])

                sig = workp.tile([P, K, CH], _F32)
                nc.scalar.activation(out=sig, in_=xt[:, :, 0:CH], func=_AF.Sigmoid)
                ex = workp.tile([P, K, CLASSES], _F32)
                nc.scalar.activation(
                    out=ex, in_=xt[:, :, CH : CH + CLASSES], func=_AF.Exp
                )

                sums = workp.tile([P, K], _F32)
                nc.vector.tensor_reduce(
                    out=sums, in_=ex, axis=mybir.AxisListType.X, op=_OP.add
                )
                rec = workp.tile([P, K], _F32)
                nc.vector.reciprocal(out=rec, in_=sums)

                sig5 = sig[:].rearrange("p k (a c) -> p k a c", a=NUM)  # [P,K,5,5]
                confc = sig5[:, :, :, COORDS]                          # [P,K,5]

                scales = workp.tile([P, K, NUM], _F32)
                nc.vector.tensor_mul(
                    out=scales,
                    in0=confc,
                    in1=rec[:, :, None].to_broadcast([P, K, NUM]),
                )

                co = clsp.tile([P, K, NUM, CLASSES], _F32)
                nc.vector.tensor_mul(
                    out=co,
                    in0=ex[:, :, None, :].to_broadcast([P, K, NUM, CLASSES]),
                    in1=scales[:, :, :, None].to_broadcast([P, K, NUM, CLASSES]),
                )
                nc.sync.dma_start(out=clsv[t], in_=co)

                # box coords: cxy = (sig(txy) + grid)/SIDE; half wh = sig(twh)/2
                pxy = workp.tile([P, K, NUM, 2], _F32)
                nc.vector.tensor_scalar_mul(
                    out=pxy, in0=sig5[:, :, :, 0:2], scalar1=1.0 / SIDE
                )
                nc.vector.tensor_add(out=pxy, in0=pxy, in1=gsb[:, t])
                half = workp.tile([P, K, NUM, 2], _F32)
                nc.vector.tensor_scalar_mul(
                    out=half, in0=sig5[:, :, :, 2:4], scalar1=0.5
                )
                lo = locp.tile([P, K, NUM, COORDS], _F32)
                nc.vector.tensor_sub(out=lo[:, :, :, 0:2], in0=pxy, in1=half)
                nc.vector.tensor_add(out=lo[:, :, :, 2:4], in0=pxy, in1=half)
                nc.sync.dma_start(out=locv[t], in_=lo)

                ct = locp.tile([P, K, NUM], _F32)
                nc.gpsimd.tensor_copy(out=ct, in_=confc)
                nc.sync.dma_start(out=confv[t], in_=ct)

    return nc


def _get_nc() -> bass.Bass:
    if "nc" not in _CACHE:
        nc = _build()
        if not nc.is_finalized():
            nc.finalize()
        _CACHE["nc"] = nc
    return _CACHE["nc"]


def run(b_x: np.ndarray, **spmd_kwargs):
    b_x = np.ascontiguousarray(np.asarray(b_x, dtype=np.float32))
    assert b_x.shape == (BATCH, FEAT), b_x.shape
    nc = _get_nc()
    gconst = _grid_const()
    in_maps = [
        {"x": b_x[i * BPC : (i + 1) * BPC], "g": gconst} for i in range(NCORES)
    ]
    res = run_bass_kernel_spmd(nc, in_maps, list(range(NCORES)), **spmd_kwargs)
    loc = np.concatenate([m["loc"] for m in res.results], axis=0)
    cls = np.concatenate([m["cls"] for m in res.results], axis=0)
    conf = np.concatenate([m["conf"] for m in res.results], axis=0)
    return (loc, cls, conf), res


def kernel(b_x: np.ndarray):
    outs, _ = run(b_x)
    return outs


# revision 9
# speedup vs baseline: 454.4650x; 454.4650x over previous
"""DetectionLayer (YOLOv2-style head) Trainium2 Bass kernel.

Input:  b_x [512, 82320] f32  (= [512, 28, 28, 105]; 105 = 5*(4+1) + 80)
Output: (b_pred_loc [512, 3920, 4], b_cls [512, 3920, 80], b_conf [512, 3920])

Sharding: pure data parallel over batch; 64 examples per NeuronCore x 8 cores.
Per-core layout: 64*784 = 50176 cells viewed as 28 tiles of [128 partitions,
14 cells, 105 ch]; every DMA row is contiguous in DRAM (>= 1120 B).
"""

import numpy as np

import concourse.bass as bass
import concourse.bacc as bacc
import concourse.tile as tile
from concourse import mybir
from concourse.bass_utils import run_bass_kernel_spmd

SIDE = 28
NUM = 5
CLASSES = 80
COORDS = 4
BATCH = 512
NCORES = 8
CH = NUM * (COORDS + 1)            # 25
FEAT = SIDE * SIDE * (CH + CLASSES)  # 82320
BPC = BATCH // NCORES              # 64 examples per core
CELLS = BPC * SIDE * SIDE          # 50176 cells per core
P = 128                            # SBUF partitions
K = 14                             # cells per partition per tile
NT = CELLS // (P * K)              # 28 tiles per core
BOXES = SIDE * SIDE * NUM          # 3920

_F32 = mybir.dt.float32
_AF = mybir.ActivationFunctionType
_OP = mybir.AluOpType

_CACHE = {}


def _grid_const() -> np.ndarray:
    # gxy[p, t, j, a, 0] = (cell col)/SIDE, [..., 1] = (cell row)/SIDE for the
    # cell at flat index t*(P*K) + p*K + j (cell index within an example
    # repeats every SIDE*SIDE). Replicated over the NUM anchors so on-device
    # access patterns stay contiguous (the BIR verifier caps compute-op APs
    # at 3 dims, which rules out stride-0 anchor broadcast here).
    n = np.arange(CELLS, dtype=np.int64)
    s = n % (SIDE * SIDE)
    gx = (s % SIDE).astype(np.float32) / SIDE
    gy = (s // SIDE).astype(np.float32) / SIDE
    gxy = np.stack([gx, gy], axis=-1)          # [CELLS, 2]
    gxy = np.repeat(gxy[:, None, :], NUM, axis=1)  # [CELLS, NUM, 2]
    gxy = gxy.reshape(NT, P, K, NUM, 2).transpose(1, 0, 2, 3, 4)
    return np.ascontiguousarray(gxy.reshape(P, NT * K * NUM * 2))


def _build(repeat: int = 1) -> bass.Bass:
    nc = bacc.Bacc("TRN2", target_bir_lowering=False)

    x = nc.dram_tensor("x", [BPC, FEAT], _F32, kind="ExternalInput")
    g = nc.dram_tensor("g", [P, NT * K * NUM * 2], _F32, kind="ExternalInput")
    loc = nc.dram_tensor("loc", [BPC, BOXES, COORDS], _F32, kind="ExternalOutput")
    cls = nc.dram_tensor("cls", [BPC, BOXES, CLASSES], _F32, kind="ExternalOutput")
    conf = nc.dram_tensor("conf", [BPC, BOXES], _F32, kind="ExternalOutput")

    xv = x[:, :].flatten().rearrange("(t p k c) -> t p k c", t=NT, p=P, k=K)
    locv = loc[:, :, :].flatten().rearrange(
        "(t p k a c) -> t p k a c", t=NT, p=P, k=K, a=NUM
    )
    clsv = cls[:, :, :].flatten().rearrange(
        "(t p k a c) -> t p k a c", t=NT, p=P, k=K, a=NUM
    )
    confv = conf[:, :].flatten().rearrange("(t p k a) -> t p k a", t=NT, p=P, k=K)
    gv = g[:, :].rearrange("p (t k a c) -> p t k a c", t=NT, k=K, a=NUM)

    with tile.TileContext(nc) as tc:
        with (
            tc.tile_pool(name="const", bufs=1) as constp,
            tc.tile_pool(name="io", bufs=3) as iop,
            tc.tile_pool(name="work", bufs=3) as workp,
            tc.tile_pool(name="clsout", bufs=3) as clsp,
            tc.tile_pool(name="locout", bufs=3) as locp,
        ):
            gsb = constp.tile([P, NT, K, NUM, 2], _F32)
            nc.sync.dma_start(out=gsb, in_=gv)

            for t in [t for _ in range(repeat) for t in range(NT)]:
                xt = iop.tile([P, K, CH + CLASSES], _F32)
                nc.sync.dma_start(out=xt, in_=xv[t])

                sig = workp.tile([P, K, CH], _F32)
                nc.scalar.activation(out=sig, in_=xt[:, :, 0:CH], func=_AF.Sigmoid)
                ex = workp.tile([P, K, CLASSES], _F32)
                nc.scalar.activation(
                    out=ex, in_=xt[:, :, CH : CH + CLASSES], func=_AF.Exp
                )

                sums = workp.tile([P, K], _F32)
                nc.vector.tensor_reduce(
                    out=sums, in_=ex, axis=mybir.AxisListType.X, op=_OP.add
                )
                rec = workp.tile([P, K], _F32)
                nc.vector.reciprocal(out=rec, in_=sums)

                sig5 = sig[:].rearrange("p k (a c) -> p k a c", a=NUM)  # [P,K,5,5]
                confc = sig5[:, :, :, COORDS]                          # [P,K,5]

                scales = workp.tile([P, K, NUM], _F32)
                nc.vector.tensor_mul(
                    out=scales,
                    in0=confc,
                    in1=rec[:, :, None].to_broadcast([P, K, NUM]),
                )

                # cls out: anchors 0-2 on DVE (tensor_tensor never grabs the
                # shared SBUF port pair), anchors 3-4 on the otherwise-idle
                # GpSimd. Per-anchor ops keep every AP <= 3D for the verifier.
                co = clsp.tile([P, K, NUM, CLASSES], _F32)
                for a in range(NUM):
                    eng = nc.vector if a < 3 else nc.gpsimd
                    eng.tensor_mul(
                        out=co[:, :, a, :],
                        in0=ex,
                        in1=scales[:, :, a][:, :, None].to_broadcast(
                            [P, K, CLASSES]
                        ),
                    )
                nc.sync.dma_start(out=clsv[t], in_=co)

                # box coords: cxy = (sig(txy) + grid)/SIDE; half wh = sig(twh)/2
                # (immediate-scale muls go to ACT to keep DVE lean)
                pxy = workp.tile([P, K, NUM, 2], _F32)
                nc.scalar.mul(out=pxy, in_=sig5[:, :, :, 0:2], mul=1.0 / SIDE)
                nc.vector.tensor_add(out=pxy, in0=pxy, in1=gsb[:, t])
                half = workp.tile([P, K, NUM, 2], _F32)
                nc.scalar.mul(out=half, in_=sig5[:, :, :, 2:4], mul=0.5)
                lo = locp.tile([P, K, NUM, COORDS], _F32)
                nc.vector.tensor_sub(out=lo[:, :, :, 0:2], in0=pxy, in1=half)
                nc.vector.tensor_add(out=lo[:, :, :, 2:4], in0=pxy, in1=half)
                nc.sync.dma_start(out=locv[t], in_=lo)

                ct = locp.tile([P, K, NUM], _F32)
                nc.gpsimd.tensor_copy(out=ct, in_=confc)
                nc.sync.dma_start(out=confv[t], in_=ct)

    return nc


def _get_nc() -> bass.Bass:
    if "nc" not in _CACHE:
        nc = _build()
        if not nc.is_finalized():
            nc.finalize()
        _CACHE["nc"] = nc
    return _CACHE["nc"]


def run(b_x: np.ndarray, **spmd_kwargs):
    b_x = np.ascontiguousarray(np.asarray(b_x, dtype=np.float32))
    assert b_x.shape == (BATCH, FEAT), b_x.shape
    nc = _get_nc()
    gconst = _grid_const()
    in_maps = [
        {"x": b_x[i * BPC : (i + 1) * BPC], "g": gconst} for i in range(NCORES)
    ]
    res = run_bass_kernel_spmd(nc, in_maps, list(range(NCORES)), **spmd_kwargs)
    loc = np.concatenate([m["loc"] for m in res.results], axis=0)
    cls = np.concatenate([m["cls"] for m in res.results], axis=0)
    conf = np.concatenate([m["conf"] for m in res.results], axis=0)
    return (loc, cls, conf), res


def kernel(b_x: np.ndarray):
    outs, _ = run(b_x)
    return outs


# revision 20
# speedup vs baseline: 465.0758x; 1.0233x over previous
"""DetectionLayer (YOLOv2-style head) Trainium2 Bass kernel.

Input:  b_x [512, 82320] f32  (= [512, 28, 28, 105]; 105 = 5*(4+1) + 80)
Output: (b_pred_loc [512, 3920, 4], b_cls [512, 3920, 80], b_conf [512, 3920])

Sharding: pure data parallel over batch; 64 examples per NeuronCore x 8 cores.
Per-core layout: 64*784 = 50176 cells viewed as 14 tiles of [128 partitions,
28 cells, 105 ch]; every DMA row is contiguous in DRAM (>= 560 B).
"""

import numpy as np

import concourse.bass as bass
import concourse.bacc as bacc
import concourse.tile as tile
from concourse import mybir
from concourse.bass_utils import run_bass_kernel_spmd

SIDE = 28
NUM = 5
CLASSES = 80
COORDS = 4
BATCH = 512
NCORES = 8
CH = NUM * (COORDS + 1)            # 25
FEAT = SIDE * SIDE * (CH + CLASSES)  # 82320
BPC = BATCH // NCORES              # 64 examples per core
CELLS = BPC * SIDE * SIDE          # 50176 cells per core
P = 128                            # SBUF partitions
K = 28                             # cells per partition per tile
NT = CELLS // (P * K)              # 14 tiles per core
BOXES = SIDE * SIDE * NUM          # 3920

_F32 = mybir.dt.float32
_AF = mybir.ActivationFunctionType
_OP = mybir.AluOpType

_CACHE = {}


def _grid_const(k: int = K) -> np.ndarray:
    # gxy[p, t, j, a, 0] = (cell col)/SIDE, [..., 1] = (cell row)/SIDE for the
    # cell at flat index t*(P*K) + p*K + j (cell index within an example
    # repeats every SIDE*SIDE). Replicated over the NUM anchors so on-device
    # access patterns stay contiguous (the BIR verifier caps compute-op APs
    # at 3 dims, which rules out stride-0 anchor broadcast here).
    nt = CELLS // (P * k)
    n = np.arange(CELLS, dtype=np.int64)
    s = n % (SIDE * SIDE)
    gx = (s % SIDE).astype(np.float32) / SIDE
    gy = (s // SIDE).astype(np.float32) / SIDE
    gxy = np.stack([gx, gy], axis=-1)          # [CELLS, 2]
    gxy = np.repeat(gxy[:, None, :], NUM, axis=1)  # [CELLS, NUM, 2]
    gxy = gxy.reshape(nt, P, k, NUM, 2).transpose(1, 0, 2, 3, 4)
    return np.ascontiguousarray(gxy.reshape(P, nt * k * NUM * 2))


def _build(
    repeat: int = 1,
    io_bufs: int = 3,
    cls_bufs: int = 2,
    k: int = K,
    cls_on_act: bool = True,
    out_on_act: bool = True,
) -> bass.Bass:
    K_, NT_ = k, CELLS // (P * k)
    nc = bacc.Bacc("TRN2", target_bir_lowering=False)

    x = nc.dram_tensor("x", [BPC, FEAT], _F32, kind="ExternalInput")
    g = nc.dram_tensor("g", [P, NT_ * K_ * NUM * 2], _F32, kind="ExternalInput")
    loc = nc.dram_tensor("loc", [BPC, BOXES, COORDS], _F32, kind="ExternalOutput")
    cls = nc.dram_tensor("cls", [BPC, BOXES, CLASSES], _F32, kind="ExternalOutput")
    conf = nc.dram_tensor("conf", [BPC, BOXES], _F32, kind="ExternalOutput")

    xv = x[:, :].flatten().rearrange("(t p k c) -> t p k c", t=NT_, p=P, k=K_)
    locv = loc[:, :, :].flatten().rearrange(
        "(t p k a c) -> t p k a c", t=NT_, p=P, k=K_, a=NUM
    )
    clsv = cls[:, :, :].flatten().rearrange(
        "(t p k a c) -> t p k a c", t=NT_, p=P, k=K_, a=NUM
    )
    confv = conf[:, :].flatten().rearrange("(t p k a) -> t p k a", t=NT_, p=P, k=K_)
    gv = g[:, :].rearrange("p (t k a c) -> p t k a c", t=NT_, k=K_, a=NUM)

    with tile.TileContext(nc) as tc:
        with (
            tc.tile_pool(name="const", bufs=1) as constp,
            tc.tile_pool(name="io", bufs=io_bufs) as iop,
            tc.tile_pool(name="work", bufs=3) as workp,
            tc.tile_pool(name="clsout", bufs=cls_bufs) as clsp,
            tc.tile_pool(name="locout", bufs=3) as locp,
        ):
            gsb = constp.tile([P, NT_, K_, NUM, 2], _F32)
            nc.scalar.dma_start(out=gsb, in_=gv)

            for t in [t for _ in range(repeat) for t in range(NT_)]:
                xt = iop.tile([P, K_, CH + CLASSES], _F32)
                nc.sync.dma_start(out=xt, in_=xv[t])

                sig = workp.tile([P, K_, CH], _F32)
                nc.scalar.activation(out=sig, in_=xt[:, :, 0:CH], func=_AF.Sigmoid)
                ex = workp.tile([P, K_, CLASSES], _F32)
                nc.scalar.activation(
                    out=ex, in_=xt[:, :, CH : CH + CLASSES], func=_AF.Exp
                )

                sums = workp.tile([P, K_], _F32)
                nc.vector.tensor_reduce(
                    out=sums, in_=ex, axis=mybir.AxisListType.X, op=_OP.add
                )
                rec = workp.tile([P, K_], _F32)
                nc.vector.reciprocal(out=rec, in_=sums)

                sig5 = sig[:].rearrange("p k (a c) -> p k a c", a=NUM)  # [P,K,5,5]
                confc = sig5[:, :, :, COORDS]                          # [P,K,5]

                scales = workp.tile([P, K_, NUM], _F32)
                nc.vector.tensor_mul(
                    out=scales,
                    in0=confc,
                    in1=rec[:, :, None].to_broadcast([P, K_, NUM]),
                )

                # cls out: anchors 0-2 on DVE (tensor_tensor never grabs the
                # shared SBUF port pair), anchors 3-4 on the otherwise-idle
                # GpSimd. Per-anchor ops keep every AP <= 3D for the verifier.
                co = clsp.tile([P, K_, NUM, CLASSES], _F32)
                for a in range(NUM):
                    eng = nc.vector if a < 3 else nc.gpsimd
                    eng.tensor_mul(
                        out=co[:, :, a, :],
                        in0=ex,
                        in1=scales[:, :, a][:, :, None].to_broadcast(
                            [P, K_, CLASSES]
                        ),
                    )
                (nc.scalar if cls_on_act else nc.sync).dma_start(
                    out=clsv[t], in_=co
                )
                out_eng = nc.scalar if out_on_act else nc.sync

                # box coords: cxy = (sig(txy) + grid)/SIDE; half wh = sig(twh)/2
                # (immediate-scale muls go to ACT to keep DVE lean)
                pxy = workp.tile([P, K_, NUM, 2], _F32)
                nc.scalar.mul(out=pxy, in_=sig5[:, :, :, 0:2], mul=1.0 / SIDE)
                nc.vector.tensor_add(out=pxy, in0=pxy, in1=gsb[:, t])
                half = workp.tile([P, K_, NUM, 2], _F32)
                nc.scalar.mul(out=half, in_=sig5[:, :, :, 2:4], mul=0.5)
                lo = locp.tile([P, K_, NUM, COORDS], _F32)
                nc.vector.tensor_sub(out=lo[:, :, :, 0:2], in0=pxy, in1=half)
                nc.vector.tensor_add(out=lo[:, :, :, 2:4], in0=pxy, in1=half)
                out_eng.dma_start(out=locv[t], in_=lo)

                ct = locp.tile([P, K_, NUM], _F32)
                nc.gpsimd.tensor_copy(out=ct, in_=confc)
                out_eng.dma_start(out=confv[t], in_=ct)

    return nc


def _get_nc() -> bass.Bass:
    if "nc" not in _CACHE:
        nc = _build()
        if not nc.is_finalized():
            nc.finalize()
        _CACHE["nc"] = nc
    return _CACHE["nc"]


def run(b_x: np.ndarray, **spmd_kwargs):
    b_x = np.ascontiguousarray(np.asarray(b_x, dtype=np.float32))
    assert b_x.shape == (BATCH, FEAT), b_x.shape
    nc = _get_nc()
    gconst = _grid_const()
    in_maps = [
        {"x": b_x[i * BPC : (i + 1) * BPC], "g": gconst} for i in range(NCORES)
    ]
    res = run_bass_kernel_spmd(nc, in_maps, list(range(NCORES)), **spmd_kwargs)
    loc = np.concatenate([m["loc"] for m in res.results], axis=0)
    cls = np.concatenate([m["cls"] for m in res.results], axis=0)
    conf = np.concatenate([m["conf"] for m in res.results], axis=0)
    return (loc, cls, conf), res


def kernel(b_x: np.ndarray):
    outs, _ = run(b_x)
    return outs
